# revision 73
# baseline (speedup 1.0000x reference)
"""Trainium2 Bass kernel for nn_Attention_54614804136573 (topk_masking).

Sharding: 8 cores = 4 batches x 2 head-groups (8 heads each). Each core gets
its batch's 8 head-chunks of x pre-transposed to [c, n] bf16, plus the
chunk-summed xsum (f32) used for the token-importance logits. It computes the
mask redundantly, runs its 8 heads of attention, and produces a partial
to_out product for its 1024-wide d-slice. The host sums the two partials per
batch and adds bo.

v5 structure:
 - logits come from the host-staged chunk-sum of x, so the serial top-k mask
   chain starts at ~3us and is fully hidden.
 - bias-folded attention: softmax(q_i.k_j) == softmax(x_i.(M x_j) + e_j)
   with M = Wq^T Wk and e_j = (Wk^T bq).x_j + bq.bk (the per-query term
   cancels in softmax). The Q projection disappears; exp(SCALE*e_j) is
   folded into the V-mask and keep-mask columns, so exp() runs with the
   constant scale only.
 - the token mask enters only via the V values and the softmax denominator
   (binary keep-mask column + a +25 correction for the masked tokens).
 - per-head software pipeline: B(h)=V proj+transpose+mask, A(h)=x.K2+exp
   two heads ahead, C(h)=PV+denominator+normalize.
"""

import sys

sys.path.insert(0, "/opt/trn_rl_repo")

import numpy as np
import ml_dtypes

import concourse.mybir as mybir
import concourse.tile as tile
from concourse import bacc, bass_utils
from concourse.masks import make_identity
from concourse.tile import add_dep_helper

B = 4
N = 1024
C = 128
D = 2048
NCHUNK = 16  # d-chunks of 128 (= patch positions = heads)
HPC = 8  # heads per core
MASK_NUM = 25
SCALE = 64.0 ** -0.5  # 0.125

F32 = mybir.dt.float32
F32R = mybir.dt.float32r
BF16 = mybir.dt.bfloat16
U32 = mybir.dt.uint32
Exp = mybir.ActivationFunctionType.Exp
Ident = mybir.ActivationFunctionType.Identity
NEG_BIG = -1e30


def _body(tc, xt_d, xsum_d, wm_d, wv_d, wke_d, eb_d, bv_d, wtc_d,
          wo_d, outT_d):
    nc = tc.nc
    mscr = nc.dram_tensor("mscr", (N,), F32, kind="Internal").ap()
    bscr = nc.dram_tensor("bscr", (N,), BF16, kind="Internal").ap()
    dscr = nc.dram_tensor("dscr", (HPC, N), F32, kind="Internal").ap()

    with (
        tc.tile_pool(name="consts", bufs=1) as consts,
        tc.tile_pool(name="persist", bufs=1) as persist,
    ):
        # ---- constants ----
        identb = consts.tile([128, 128], BF16)
        make_identity(nc, identb)


        # ---- persistent activations ----
        k2T = persist.tile([128, HPC, N], BF16)  # [c', h, n] 2 MB
        vnat = persist.tile([128, HPC, 8, C], BF16)  # [j, h, jt, c] 2 MB
        outT_sb = persist.tile([128, HPC, N], BF16)  # [c, h, i] 2 MB
        woT_sb = persist.tile([128, HPC, D], BF16)  # [d, h-chunk, o] 4 MB
        xb = persist.tile([128, HPC, N], BF16)  # [c, k, n] own chunks, 2 MB
        mask_col = persist.tile([128, 8], F32)
        bkm_col = persist.tile([128, 8], BF16)
        expE = persist.tile([128, HPC, 8], BF16)
        mh = persist.tile([128, HPC, 8], F32)    # mask * exp(SCALE*e)
        bkmh = persist.tile([128, HPC, 8], BF16)  # keepmask * exp(SCALE*e)

        # xsum + x on the sync queue (transfers are serialized on the shared
        # DMA engines anyway); weights on the gpsimd queue; the scalar (Act)
        # queue carries NO early descriptor-gen so the mask chain's exp can
        # start immediately.
        xsum_sb = consts.tile([128, N], F32R)
        nc.sync.dma_start(out=xsum_sb[:, 0:512], in_=xsum_d[:, 0:512])
        nc.scalar.dma_start(out=xsum_sb[:, 512:1024], in_=xsum_d[:, 512:1024])
        wtc_sb = consts.tile([C, 1], F32R)
        nc.gpsimd.dma_start(out=wtc_sb, in_=wtc_d)
        wm_sb = consts.tile([C, C], BF16)
        nc.gpsimd.dma_start(out=wm_sb, in_=wm_d)
        wke_sb = consts.tile([C, 32], BF16)
        nc.gpsimd.dma_start(out=wke_sb, in_=wke_d)
        eb_sb = consts.tile([128, 1], F32)
        nc.gpsimd.dma_start(out=eb_sb, in_=eb_d)
        wv_sb = consts.tile([C, C], BF16)
        nc.gpsimd.dma_start(out=wv_sb, in_=wv_d)
        bv_sb = consts.tile([C, 1], F32)
        nc.gpsimd.dma_start(out=bv_sb, in_=bv_d)

        x_dmas = []
        for k in range(HPC):
            x_dmas.append(nc.sync.dma_start(out=xb[:, k, :], in_=xt_d[k]))

        # ====== logits + mask chain (starts immediately) ===================
        with (
            tc.tile_pool(name="lg_psum", bufs=1, space="PSUM") as lg_psum,
            tc.tile_pool(name="mrows", bufs=1) as mrows,
        ):
            lg = lg_psum.tile([1, N], F32)
            for half in range(2):
                nc.tensor.matmul(
                    lg[:, half * 512 : (half + 1) * 512],
                    wtc_sb,
                    xsum_sb[:, half * 512 : (half + 1) * 512],
                    start=True, stop=True,
                )
            smrow = mrows.tile([1, N], F32)
            ssum = mrows.tile([1, 1], F32)
            nc.scalar.activation(out=smrow, in_=lg, func=Exp, accum_out=ssum)
            srecip = mrows.tile([1, 1], F32)
            nc.vector.reciprocal(srecip, ssum)
            # softmax values via Act (parallel with the DVE top-k rounds)
            nc.scalar.activation(
                out=smrow, in_=smrow, func=Ident, scale=srecip
            )
            negrow = mrows.tile([1, N], F32)
            nc.scalar.activation(out=negrow, in_=lg, func=Ident, scale=-1.0)
            scratch = mrows.tile([1, N], F32)
            m8 = mrows.tile([1, 8], F32)
            for r in range(3):
                nc.vector.max(out=m8, in_=negrow if r == 0 else scratch)
                nc.vector.match_replace(
                    out=scratch, in_to_replace=m8,
                    in_values=negrow if r == 0 else scratch,
                    imm_value=NEG_BIG,
                )
            nc.vector.max(out=m8, in_=scratch)  # m8[0,0] = -(25th smallest)
            ind = mrows.tile([1, N], F32)
            nc.vector.tensor_scalar(
                ind, negrow, m8[:, 0:1], None, op0=mybir.AluOpType.is_lt
            )
            # keep-mask row in bf16 for the denominator contraction
            bkm_row = mrows.tile([1, N], BF16)
            nc.gpsimd.tensor_copy(bkm_row, ind)
            # mask = max(indicator, softmax) : softmax values are <= 1
            nc.vector.tensor_tensor(
                out=smrow, in0=ind, in1=smrow, op=mybir.AluOpType.max
            )
            w_m = nc.sync.dma_start(out=mscr, in_=smrow)
            r_m = nc.sync.dma_start(
                out=mask_col, in_=mscr.rearrange("(t j) -> j t", j=128)
            )
            add_dep_helper(r_m.ins, w_m.ins, sync=True, reason="mask RAW")
            w_b = nc.scalar.dma_start(out=bscr, in_=bkm_row)
            r_b = nc.scalar.dma_start(
                out=bkm_col, in_=bscr.rearrange("(t j) -> j t", j=128)
            )
            add_dep_helper(r_b.ins, w_b.ins, sync=True, reason="bkm RAW")
            # Wo is only needed in phase 3 -- keep its transfers off the DMA
            # engines until the x chunks and mask roundtrips are through.
            for h in range(HPC):
                w_inst = nc.sync.dma_start(
                    out=woT_sb[:, h, :], in_=wo_d[h * 128 : (h + 1) * 128, :]
                )
                add_dep_helper(
                    w_inst.ins, r_b.ins, sync=True,
                    reason="defer woT behind mask roundtrips",
                )

            # ====== K2 projections + e rows (interleaved by chunk arrival) =
            with (
                tc.tile_pool(name="mm_psum", bufs=2, space="PSUM") as mm_psum,
                tc.tile_pool(name="e4_psum", bufs=1, space="PSUM") as e4_psum,
            ):
                identf = consts.tile([128, 128], F32)
                make_identity(nc, identf)
                esb = consts.tile([128, 2, N], F32)
                etr = consts.tile([128, 2, 8, 128], F32)

                def emit_K2(h):
                    pp = mm_psum.tile([128, N], F32)
                    for half in range(2):
                        nc.tensor.matmul(
                            pp[:, half * 512 : (half + 1) * 512],
                            wm_sb,
                            xb[:, h, half * 512 : (half + 1) * 512],
                            start=True, stop=True,
                        )
                    if h >= 6:
                        # k2T h6/h7 aren't needed until much later; draining
                        # them on DVE (free once the mask rounds end) lets
                        # Act reach the first exp() stream ~2us sooner.
                        nc.vector.tensor_copy(k2T[:, h, :], pp)
                    else:
                        nc.scalar.activation(
                            out=k2T[:, h, :], in_=pp, func=Ident
                        )

                def emit_e_mm(g):
                    # 4 heads packed per PSUM tile (partitions 0/32/64/96);
                    # stationary zero-padded to 32 cols so all partitions are
                    # written. Row 32m holds head (4g+m)'s e values. The row
                    # -> column transpose happens on the PE (no DMA).
                    e4 = e4_psum.tile([128, N], F32, tag="e4")
                    for m in range(4):
                        h = g * 4 + m
                        for half in range(2):
                            nc.tensor.matmul(
                                e4[32 * m : 32 * m + 32,
                                   half * 512 : (half + 1) * 512],
                                wke_sb,
                                xb[:, h, half * 512 : (half + 1) * 512],
                                start=True, stop=True,
                                tile_position=(0, 32 * m),
                            )
                    nc.scalar.activation(out=esb[:, g, :], in_=e4, func=Ident)

                def emit_e_tr(g):
                    etp = e4_psum.tile([128, 8, 128], F32, tag="e4")
                    for t in range(8):
                        nc.tensor.transpose(
                            etp[:, t, :], esb[:, g, t * 128 : (t + 1) * 128],
                            identf,
                        )
                    nc.scalar.activation(out=etr[:, g, :, :], in_=etp,
                                         func=Ident)
                    # expE = exp(SCALE*e + SCALE*eb)
                    for m in range(4):
                        h = g * 4 + m
                        nc.scalar.activation(
                            out=expE[:, h, :], in_=etr[:, g, :, 32 * m],
                            func=Exp, scale=SCALE, bias=eb_sb,
                        )

                emit_K2(0)
                emit_K2(1)
                emit_K2(2)
                emit_e_mm(0)
                emit_K2(3)
                emit_K2(4)
                emit_e_tr(0)
                emit_K2(5)
                emit_K2(6)
                emit_e_mm(1)
                emit_K2(7)
                emit_e_tr(1)

                for h in range(HPC):
                    nc.gpsimd.tensor_tensor(
                        out=mh[:, h, :], in0=mask_col, in1=expE[:, h, :],
                        op=mybir.AluOpType.mult,
                    )
                    nc.gpsimd.tensor_tensor(
                        out=bkmh[:, h, :], in0=bkm_col, in1=expE[:, h, :],
                        op=mybir.AluOpType.mult,
                    )

        # ================= phase 2: attention ==============================
        vT_pool = tc.tile_pool(name="vT", bufs=2)
        vT = vT_pool.__enter__()
        attn_pools = (
            tc.tile_pool(name="pexp", bufs=26),
            tc.tile_pool(name="dvp", bufs=2),
            tc.tile_pool(name="st_psum", bufs=2, space="PSUM"),
            tc.tile_pool(name="ot_psum", bufs=1, space="PSUM"),
            tc.tile_pool(name="dn_psum", bufs=2, space="PSUM"),
        )
        pexp, dvp, st_psum, ot_psum, dn_psum = (
            p.__enter__() for p in attn_pools
        )
        pexp_tiles = {}  # (h, jt) -> tile

        def emit_A_part(h, jt):
            st = st_psum.tile([128, N], F32, tag="st")
            for half in range(2):
                nc.tensor.matmul(
                    st[:, half * 512 : (half + 1) * 512],
                    k2T[:, h, jt * 128 : (jt + 1) * 128],
                    xb[:, h, half * 512 : (half + 1) * 512],
                    start=True, stop=True,
                )
            pexp_t = pexp.tile([128, N], BF16)
            nc.scalar.activation(out=pexp_t, in_=st, func=Exp, scale=SCALE)
            pexp_tiles[(h, jt)] = pexp_t

        def emit_A(h):
            for jt in range(8):
                emit_A_part(h, jt)

        def emit_B(h):
            """V projection + transpose + (mask*expE) multiply for head h."""
            pp = st_psum.tile([128, N], F32, tag="st")
            for half in range(2):
                nc.tensor.matmul(
                    pp[:, half * 512 : (half + 1) * 512],
                    wv_sb,
                    xb[:, h, half * 512 : (half + 1) * 512],
                    start=True, stop=True,
                )
            vT_h = vT.tile([128, N], BF16)
            nc.vector.tensor_scalar(
                vT_h, pp, bv_sb, None, op0=mybir.AluOpType.add
            )
            pv8 = st_psum.tile([128, 8, 128], BF16, tag="st")
            for jt in range(8):
                nc.tensor.transpose(
                    pv8[:, jt, :], vT_h[:, jt * 128 : (jt + 1) * 128],
                    identb,
                )
            nc.vector.tensor_tensor(
                out=vnat[:, h, :, :],
                in0=pv8,
                in1=mh[:, h, :].unsqueeze(-1).broadcast_to([128, 8, 128]),
                op=mybir.AluOpType.mult,
            )

        def emit_C_mm(h, interleave=None):
            """PV + masked denominator matmuls for head h."""
            ot = ot_psum.tile([128, N], F32)
            dn0 = dn_psum.tile([1, 512], F32, tag="dn")
            dn1 = dn_psum.tile([1, 512], F32, tag="dn")
            dnh = [dn0, dn1]
            for jt in range(8):
                pexp_t = pexp_tiles.pop((h, jt))
                for half in range(2):
                    nc.tensor.matmul(
                        ot[:, half * 512 : (half + 1) * 512],
                        vnat[:, h, jt, :],
                        pexp_t[:, half * 512 : (half + 1) * 512],
                        start=(jt == 0), stop=(jt == 7),
                    )
                for half in range(2):
                    nc.tensor.matmul(
                        dnh[half][:, :],
                        bkmh[:, h, jt : jt + 1],
                        pexp_t[:, half * 512 : (half + 1) * 512],
                        start=(jt == 0), stop=(jt == 7),
                    )
                if interleave is not None:
                    interleave(jt)
            return ot, dnh

        def emit_C_norm(h, ot, dnh):
            nc.vector.tensor_copy(outT_sb[:, h, :], ot)
            rsum = dvp.tile([1, N], F32, tag="rsum")
            # masked tokens contribute exp(~0)=1 each to the denominator;
            # per-half reads so each dn bank frees as soon as it stops.
            for half in range(2):
                nc.vector.tensor_scalar(
                    rsum[:, half * 512 : (half + 1) * 512], dnh[half],
                    float(MASK_NUM), None, op0=mybir.AluOpType.add,
                )
            rrow = dvp.tile([1, N], F32, tag="rrow")
            nc.vector.reciprocal(rrow, rsum)
            w_i = nc.sync.dma_start(out=dscr[h, :], in_=rrow)
            rb_sb = dvp.tile([128, N], F32)
            r_i = nc.sync.dma_start(
                out=rb_sb, in_=dscr[h, :].partition_broadcast(128)
            )
            add_dep_helper(r_i.ins, w_i.ins, sync=True, reason="recip RAW")
            nc.vector.tensor_mul(outT_sb[:, h, :], outT_sb[:, h, :], rb_sb)

        emit_B(0)
        emit_A(0)
        emit_B(1)
        emit_A(1)
        for h in range(HPC):
            nxt = h + 2
            if nxt < HPC:
                ot, dnh = emit_C_mm(
                    h, interleave=lambda jt, h2=nxt: emit_A_part(h2, jt)
                )
                emit_B(nxt)
            else:
                ot, dnh = emit_C_mm(h)
            emit_C_norm(h, ot, dnh)

        for p in reversed(attn_pools):
            p.__exit__(None, None, None)
        vT_pool.__exit__(None, None, None)

        # ============= phase 3: to_out partial =============================
        with (
            tc.tile_pool(name="fo_psum", bufs=3, space="PSUM") as fo_psum,
            tc.tile_pool(name="fout", bufs=4) as fout_pool,
        ):
            def finish_oc(oc, fo, last=False):
                if not last:
                    for half in range(2):
                        nc.tensor.matmul(
                            fo[:, half * 512 : (half + 1) * 512],
                            woT_sb[:, HPC - 1, oc * 128 : (oc + 1) * 128],
                            outT_sb[:, HPC - 1,
                                    half * 512 : (half + 1) * 512],
                            start=False, stop=True,
                        )
                fout = fout_pool.tile([128, N], BF16)
                if last:
                    # quarter-split drain + store so the tail is short
                    engs = (nc.sync, nc.gpsimd, nc.sync, nc.gpsimd)
                    for q in range(4):
                        sl = slice(q * 256, (q + 1) * 256)
                        if q % 2 == 0:
                            nc.vector.tensor_copy(fout[:, sl], fo[:, sl])
                        else:
                            nc.scalar.activation(
                                out=fout[:, sl], in_=fo[:, sl], func=Ident
                            )
                        engs[q].dma_start(
                            out=outT_d[oc * 128 : (oc + 1) * 128, sl],
                            in_=fout[:, sl],
                        )
                    return
                if oc % 2 == 0:
                    nc.vector.tensor_copy(fout, fo)
                else:
                    nc.scalar.activation(out=fout, in_=fo, func=Ident)
                for sh in range(2):
                    eng = nc.sync if sh % 2 == 0 else nc.scalar
                    eng.dma_start(
                        out=outT_d[oc * 128 : (oc + 1) * 128,
                                   sh * 512 : (sh + 1) * 512],
                        in_=fout[:, sh * 512 : (sh + 1) * 512],
                    )

            pending_oc = None
            for oc in range(16):
                fo = fo_psum.tile([128, N], F32)
                if oc == 15 and pending_oc is not None:
                    # flush early so its drain overlaps the last matmuls
                    finish_oc(*pending_oc)
                    pending_oc = None
                nh = HPC if oc == 15 else HPC - 1
                for half in range(2):
                    for h in range(nh):
                        nc.tensor.matmul(
                            fo[:, half * 512 : (half + 1) * 512],
                            woT_sb[:, h, oc * 128 : (oc + 1) * 128],
                            outT_sb[:, h, half * 512 : (half + 1) * 512],
                            start=(h == 0), stop=(h == HPC - 1),
                        )
                if pending_oc is not None:
                    finish_oc(*pending_oc)
                pending_oc = (oc, fo)
            finish_oc(*pending_oc, last=True)


_CACHE = {}


def _get_module():
    if "nc" in _CACHE:
        return _CACHE["nc"]
    nc = bacc.Bacc("TRN2", target_bir_lowering=False, debug=False, num_devices=8)
    xt_d = nc.dram_tensor("xt", (HPC, 128, N), BF16, kind="ExternalInput").ap()
    xsum_d = nc.dram_tensor("xsum", (128, N), F32R, kind="ExternalInput").ap()
    wm_d = nc.dram_tensor("wm", (C, C), BF16, kind="ExternalInput").ap()
    wv_d = nc.dram_tensor("wvT", (C, C), BF16, kind="ExternalInput").ap()
    wke_d = nc.dram_tensor("wke", (C, 32), BF16, kind="ExternalInput").ap()
    eb_d = nc.dram_tensor("eb", (128, 1), F32, kind="ExternalInput").ap()
    bv_d = nc.dram_tensor("bv", (C, 1), F32, kind="ExternalInput").ap()
    wtc_d = nc.dram_tensor("wtc", (C, 1), F32R, kind="ExternalInput").ap()
    wo_d = nc.dram_tensor("woT", (HPC * C, D), BF16, kind="ExternalInput").ap()
    outT_d = nc.dram_tensor("outT", (D, N), BF16, kind="ExternalOutput").ap()

    with tile.TileContext(nc) as tc:
        _body(tc, xt_d, xsum_d, wm_d, wv_d, wke_d, eb_d, bv_d,
              wtc_d, wo_d, outT_d)
    nc.compile()
    _CACHE["nc"] = nc
    return nc


def make_in_maps(x, Wq, bq, Wk, bk, Wv, bv, Wl, bl, Wo, bo):
    x = np.asarray(x, np.float32)
    Wq = np.asarray(Wq, np.float32)
    Wk = np.asarray(Wk, np.float32)
    Wv = np.asarray(Wv, np.float32)
    Wl = np.asarray(Wl, np.float32)
    Wo = np.asarray(Wo, np.float32)
    bq_ = np.asarray(bq, np.float32)
    bk_ = np.asarray(bk, np.float32)
    we = (Wl[0] @ Wq) / float(NCHUNK)  # (128,) logits weight per chunk
    common = {
        # bias-folded attention: K2 = (Wq^T Wk) x, e = (Wk^T bq).x + bq.bk
        "wm": np.ascontiguousarray(Wk.T @ Wq).astype(ml_dtypes.bfloat16),
        "wvT": np.ascontiguousarray(Wv.T).astype(ml_dtypes.bfloat16),
        "wke": np.concatenate(
            [(Wk.T @ bq_).reshape(C, 1), np.zeros((C, 31), np.float32)],
            axis=1,
        ).astype(ml_dtypes.bfloat16),
        "eb": np.full((128, 1), float(bq_ @ bk_) * SCALE, np.float32),
        "bv": np.asarray(bv, np.float32).reshape(C, 1),
        "wtc": we.astype(np.float32).reshape(C, 1),
    }
    woT = np.ascontiguousarray(Wo.T)  # (d, o)
    woT_half = [
        woT[0:1024, :].astype(ml_dtypes.bfloat16),
        woT[1024:2048, :].astype(ml_dtypes.bfloat16),
    ]
    in_maps = []
    for core in range(8):
        b, g = divmod(core, 2)
        xtb = np.ascontiguousarray(x[b].T).reshape(NCHUNK, 128, N)
        xsum = xtb.sum(axis=0)  # (128, N) f32: chunk-summed x for logits
        own = xtb[g * 8 : g * 8 + 8].astype(ml_dtypes.bfloat16)
        in_maps.append({
            "xt": np.ascontiguousarray(own),
            "xsum": np.ascontiguousarray(xsum),
            "woT": woT_half[g],
            **common,
        })
    return in_maps


def run_spmd(in_maps, trace=False, **kw):
    nc = _get_module()
    return bass_utils.run_bass_kernel_spmd(
        nc, in_maps, core_ids=list(range(8)), trace=trace, **kw
    )


def gather(results, bo):
    bo = np.asarray(bo, np.float32)
    out = np.empty((B, N, D), np.float32)
    for b in range(B):
        p0 = results[2 * b]["outT"].astype(np.float32).T
        p1 = results[2 * b + 1]["outT"].astype(np.float32).T
        out[b] = p0 + p1 + bo
    return out


def kernel(x, Wq, bq, Wk, bk, Wv, bv, Wl, bl, Wo, bo, stage=None, **_unused):
    in_maps = make_in_maps(x, Wq, bq, Wk, bk, Wv, bv, Wl, bl, Wo, bo)
    try:
        res = run_spmd(in_maps)
    except Exception:
        # transient device/runtime hiccup: retry once after a short pause
        import time as _time

        _time.sleep(2.0)
        res = run_spmd(in_maps)
    return gather(res.results, bo)


# revision 74
# speedup vs baseline: 1.0244x; 1.0244x over previous
"""Trainium2 Bass kernel for nn_Attention_54614804136573 (topk_masking).

Sharding: 8 cores = 4 batches x 2 head-groups (8 heads each). Each core gets
its batch's 8 head-chunks of x pre-transposed to [c, n] bf16, plus the
chunk-summed xsum (f32) used for the token-importance logits. It computes the
mask redundantly, runs its 8 heads of attention, and produces a partial
to_out product for its 1024-wide d-slice. The host sums the two partials per
batch and adds bo.

v5 structure:
 - logits come from the host-staged chunk-sum of x, so the serial top-k mask
   chain starts at ~3us and is fully hidden.
 - bias-folded attention: softmax(q_i.k_j) == softmax(x_i.(M x_j) + e_j)
   with M = Wq^T Wk and e_j = (Wk^T bq).x_j + bq.bk (the per-query term
   cancels in softmax). The Q projection disappears; exp(SCALE*e_j) is
   folded into the V-mask and keep-mask columns, so exp() runs with the
   constant scale only.
 - the token mask enters only via the V values and the softmax denominator
   (binary keep-mask column + a +25 correction for the masked tokens).
 - per-head software pipeline: B(h)=V proj+transpose+mask, A(h)=x.K2+exp
   two heads ahead, C(h)=PV+denominator+normalize.
"""

import sys

sys.path.insert(0, "/opt/trn_rl_repo")

import numpy as np
import ml_dtypes

import concourse.mybir as mybir
import concourse.tile as tile
from concourse import bacc, bass_utils
from concourse.masks import make_identity
from concourse.tile import add_dep_helper

B = 4
N = 1024
C = 128
D = 2048
NCHUNK = 16  # d-chunks of 128 (= patch positions = heads)
HPC = 8  # heads per core
MASK_NUM = 25
SCALE = 64.0 ** -0.5  # 0.125

F32 = mybir.dt.float32
F32R = mybir.dt.float32r
BF16 = mybir.dt.bfloat16
U32 = mybir.dt.uint32
Exp = mybir.ActivationFunctionType.Exp
Ident = mybir.ActivationFunctionType.Identity
NEG_BIG = -1e30


def _body(tc, xt_d, xsum_d, wm_d, wv_d, wke_d, eb_d, bv_d, wtc_d,
          wo_d, outT_d):
    nc = tc.nc
    mscr = nc.dram_tensor("mscr", (N,), F32, kind="Internal").ap()
    bscr = nc.dram_tensor("bscr", (N,), BF16, kind="Internal").ap()
    dscr = nc.dram_tensor("dscr", (HPC, N), F32, kind="Internal").ap()

    with (
        tc.tile_pool(name="consts", bufs=1) as consts,
        tc.tile_pool(name="persist", bufs=1) as persist,
    ):
        # ---- constants ----
        identb = consts.tile([128, 128], BF16)
        make_identity(nc, identb)


        # ---- persistent activations ----
        k2T = persist.tile([128, HPC, N], BF16)  # [c', h, n] 2 MB
        vnat = persist.tile([128, HPC, 8, C], BF16)  # [j, h, jt, c] 2 MB
        outT_sb = persist.tile([128, HPC, N], BF16)  # [c, h, i] 2 MB
        woT_sb = persist.tile([128, HPC, D], BF16)  # [d, h-chunk, o] 4 MB
        xb = persist.tile([128, HPC, N], BF16)  # [c, k, n] own chunks, 2 MB
        mask_col = persist.tile([128, 8], F32)
        bkm_col = persist.tile([128, 8], BF16)
        expE = persist.tile([128, HPC, 8], BF16)
        mh = persist.tile([128, HPC, 8], F32)    # mask * exp(SCALE*e)
        bkmh = persist.tile([128, HPC, 8], BF16)  # keepmask * exp(SCALE*e)

        # xsum + x on the sync queue (transfers are serialized on the shared
        # DMA engines anyway); weights on the gpsimd queue; the scalar (Act)
        # queue carries NO early descriptor-gen so the mask chain's exp can
        # start immediately.
        xsum_sb = consts.tile([128, N], F32R)
        nc.sync.dma_start(out=xsum_sb[:, 0:512], in_=xsum_d[:, 0:512])
        nc.scalar.dma_start(out=xsum_sb[:, 512:1024], in_=xsum_d[:, 512:1024])
        wtc_sb = consts.tile([C, 1], F32R)
        nc.gpsimd.dma_start(out=wtc_sb, in_=wtc_d)
        wm_sb = consts.tile([C, C], BF16)
        nc.gpsimd.dma_start(out=wm_sb, in_=wm_d)
        wke_sb = consts.tile([C, 32], BF16)
        nc.gpsimd.dma_start(out=wke_sb, in_=wke_d)
        eb_sb = consts.tile([128, 1], F32)
        nc.gpsimd.dma_start(out=eb_sb, in_=eb_d)
        wv_sb = consts.tile([C, C], BF16)
        nc.gpsimd.dma_start(out=wv_sb, in_=wv_d)
        bv_sb = consts.tile([C, 1], F32)
        nc.gpsimd.dma_start(out=bv_sb, in_=bv_d)

        x_dmas = []
        for k in range(HPC):
            x_dmas.append(nc.sync.dma_start(out=xb[:, k, :], in_=xt_d[k]))

        # ====== logits + mask chain (starts immediately) ===================
        with (
            tc.tile_pool(name="lg_psum", bufs=1, space="PSUM") as lg_psum,
            tc.tile_pool(name="mrows", bufs=1) as mrows,
        ):
            lg = lg_psum.tile([1, N], F32)
            for half in range(2):
                nc.tensor.matmul(
                    lg[:, half * 512 : (half + 1) * 512],
                    wtc_sb,
                    xsum_sb[:, half * 512 : (half + 1) * 512],
                    start=True, stop=True,
                )
            smrow = mrows.tile([1, N], F32)
            ssum = mrows.tile([1, 1], F32)
            nc.scalar.activation(out=smrow, in_=lg, func=Exp, accum_out=ssum)
            srecip = mrows.tile([1, 1], F32)
            nc.vector.reciprocal(srecip, ssum)
            # softmax values via Act (parallel with the DVE top-k rounds)
            nc.scalar.activation(
                out=smrow, in_=smrow, func=Ident, scale=srecip
            )
            negrow = mrows.tile([1, N], F32)
            nc.scalar.activation(out=negrow, in_=lg, func=Ident, scale=-1.0)
            scratch = mrows.tile([1, N], F32)
            m8 = mrows.tile([1, 8], F32)
            for r in range(3):
                nc.vector.max(out=m8, in_=negrow if r == 0 else scratch)
                nc.vector.match_replace(
                    out=scratch, in_to_replace=m8,
                    in_values=negrow if r == 0 else scratch,
                    imm_value=NEG_BIG,
                )
            nc.vector.max(out=m8, in_=scratch)  # m8[0,0] = -(25th smallest)
            ind = mrows.tile([1, N], F32)
            nc.vector.tensor_scalar(
                ind, negrow, m8[:, 0:1], None, op0=mybir.AluOpType.is_lt
            )
            # keep-mask row in bf16 for the denominator contraction
            bkm_row = mrows.tile([1, N], BF16)
            nc.gpsimd.tensor_copy(bkm_row, ind)
            # mask = max(indicator, softmax) : softmax values are <= 1
            nc.vector.tensor_tensor(
                out=smrow, in0=ind, in1=smrow, op=mybir.AluOpType.max
            )
            w_m = nc.sync.dma_start(out=mscr, in_=smrow)
            r_m = nc.sync.dma_start(
                out=mask_col, in_=mscr.rearrange("(t j) -> j t", j=128)
            )
            add_dep_helper(r_m.ins, w_m.ins, sync=True, reason="mask RAW")
            w_b = nc.scalar.dma_start(out=bscr, in_=bkm_row)
            r_b = nc.scalar.dma_start(
                out=bkm_col, in_=bscr.rearrange("(t j) -> j t", j=128)
            )
            add_dep_helper(r_b.ins, w_b.ins, sync=True, reason="bkm RAW")
            # Wo is only needed in phase 3 -- keep its transfers off the DMA
            # engines until the x chunks and mask roundtrips are through.
            for h in range(HPC):
                w_inst = nc.sync.dma_start(
                    out=woT_sb[:, h, :], in_=wo_d[h * 128 : (h + 1) * 128, :]
                )
                add_dep_helper(
                    w_inst.ins, r_b.ins, sync=True,
                    reason="defer woT behind mask roundtrips",
                )

            # ====== K2 projections + e rows (interleaved by chunk arrival) =
            with (
                tc.tile_pool(name="mm_psum", bufs=2, space="PSUM") as mm_psum,
                tc.tile_pool(name="e4_psum", bufs=1, space="PSUM") as e4_psum,
            ):
                identf = consts.tile([128, 128], F32)
                make_identity(nc, identf)
                esb = consts.tile([128, 2, N], F32)
                etr = consts.tile([128, 2, 8, 128], F32)

                def emit_K2(h):
                    pp = mm_psum.tile([128, N], F32)
                    for half in range(2):
                        nc.tensor.matmul(
                            pp[:, half * 512 : (half + 1) * 512],
                            wm_sb,
                            xb[:, h, half * 512 : (half + 1) * 512],
                            start=True, stop=True,
                        )
                    nc.scalar.activation(
                        out=k2T[:, h, :], in_=pp, func=Ident
                    )

                def emit_e_mm(g):
                    # 4 heads packed per PSUM tile (partitions 0/32/64/96);
                    # stationary zero-padded to 32 cols so all partitions are
                    # written. Row 32m holds head (4g+m)'s e values. The row
                    # -> column transpose happens on the PE (no DMA).
                    e4 = e4_psum.tile([128, N], F32, tag="e4")
                    for m in range(4):
                        h = g * 4 + m
                        for half in range(2):
                            nc.tensor.matmul(
                                e4[32 * m : 32 * m + 32,
                                   half * 512 : (half + 1) * 512],
                                wke_sb,
                                xb[:, h, half * 512 : (half + 1) * 512],
                                start=True, stop=True,
                                tile_position=(0, 32 * m),
                            )
                    nc.scalar.activation(out=esb[:, g, :], in_=e4, func=Ident)

                def emit_e_tr(g):
                    etp = e4_psum.tile([128, 8, 128], F32, tag="e4")
                    for t in range(8):
                        nc.tensor.transpose(
                            etp[:, t, :], esb[:, g, t * 128 : (t + 1) * 128],
                            identf,
                        )
                    nc.scalar.activation(out=etr[:, g, :, :], in_=etp,
                                         func=Ident)
                    # expE = exp(SCALE*e + SCALE*eb)
                    for m in range(4):
                        h = g * 4 + m
                        nc.scalar.activation(
                            out=expE[:, h, :], in_=etr[:, g, :, 32 * m],
                            func=Exp, scale=SCALE, bias=eb_sb,
                        )

                emit_K2(0)
                emit_K2(1)
                emit_K2(2)
                emit_e_mm(0)
                emit_K2(3)
                emit_K2(4)
                emit_e_tr(0)
                emit_K2(5)
                emit_K2(6)
                emit_e_mm(1)
                emit_K2(7)
                emit_e_tr(1)

                for h in range(HPC):
                    nc.gpsimd.tensor_tensor(
                        out=mh[:, h, :], in0=mask_col, in1=expE[:, h, :],
                        op=mybir.AluOpType.mult,
                    )
                    nc.gpsimd.tensor_tensor(
                        out=bkmh[:, h, :], in0=bkm_col, in1=expE[:, h, :],
                        op=mybir.AluOpType.mult,
                    )

        # ================= phase 2: attention ==============================
        vT_pool = tc.tile_pool(name="vT", bufs=2)
        vT = vT_pool.__enter__()
        attn_pools = (
            tc.tile_pool(name="pexp", bufs=26),
            tc.tile_pool(name="dvp", bufs=2),
            tc.tile_pool(name="st_psum", bufs=2, space="PSUM"),
            tc.tile_pool(name="ot_psum", bufs=1, space="PSUM"),
            tc.tile_pool(name="dn_psum", bufs=2, space="PSUM"),
        )
        pexp, dvp, st_psum, ot_psum, dn_psum = (
            p.__enter__() for p in attn_pools
        )
        pexp_tiles = {}  # (h, jt) -> tile

        def emit_A_part(h, jt):
            st = st_psum.tile([128, N], F32, tag="st")
            for half in range(2):
                nc.tensor.matmul(
                    st[:, half * 512 : (half + 1) * 512],
                    k2T[:, h, jt * 128 : (jt + 1) * 128],
                    xb[:, h, half * 512 : (half + 1) * 512],
                    start=True, stop=True,
                )
            pexp_t = pexp.tile([128, N], BF16)
            nc.scalar.activation(out=pexp_t, in_=st, func=Exp, scale=SCALE)
            pexp_tiles[(h, jt)] = pexp_t

        def emit_A(h):
            for jt in range(8):
                emit_A_part(h, jt)

        def emit_B(h):
            """V projection + transpose + (mask*expE) multiply for head h."""
            pp = st_psum.tile([128, N], F32, tag="st")
            for half in range(2):
                nc.tensor.matmul(
                    pp[:, half * 512 : (half + 1) * 512],
                    wv_sb,
                    xb[:, h, half * 512 : (half + 1) * 512],
                    start=True, stop=True,
                )
            vT_h = vT.tile([128, N], BF16)
            nc.vector.tensor_scalar(
                vT_h, pp, bv_sb, None, op0=mybir.AluOpType.add
            )
            pv8 = st_psum.tile([128, 8, 128], BF16, tag="st")
            for jt in range(8):
                nc.tensor.transpose(
                    pv8[:, jt, :], vT_h[:, jt * 128 : (jt + 1) * 128],
                    identb,
                )
            nc.vector.tensor_tensor(
                out=vnat[:, h, :, :],
                in0=pv8,
                in1=mh[:, h, :].unsqueeze(-1).broadcast_to([128, 8, 128]),
                op=mybir.AluOpType.mult,
            )

        def emit_C_mm(h, interleave=None):
            """PV + masked denominator matmuls for head h."""
            ot = ot_psum.tile([128, N], F32)
            dn0 = dn_psum.tile([1, 512], F32, tag="dn")
            dn1 = dn_psum.tile([1, 512], F32, tag="dn")
            dnh = [dn0, dn1]
            for jt in range(8):
                pexp_t = pexp_tiles.pop((h, jt))
                for half in range(2):
                    nc.tensor.matmul(
                        ot[:, half * 512 : (half + 1) * 512],
                        vnat[:, h, jt, :],
                        pexp_t[:, half * 512 : (half + 1) * 512],
                        start=(jt == 0), stop=(jt == 7),
                    )
                for half in range(2):
                    nc.tensor.matmul(
                        dnh[half][:, :],
                        bkmh[:, h, jt : jt + 1],
                        pexp_t[:, half * 512 : (half + 1) * 512],
                        start=(jt == 0), stop=(jt == 7),
                    )
                if interleave is not None:
                    interleave(jt)
            return ot, dnh

        def emit_C_norm(h, ot, dnh):
            nc.vector.tensor_copy(outT_sb[:, h, :], ot)
            rsum = dvp.tile([1, N], F32, tag="rsum")
            # masked tokens contribute exp(~0)=1 each to the denominator;
            # per-half reads so each dn bank frees as soon as it stops.
            for half in range(2):
                nc.vector.tensor_scalar(
                    rsum[:, half * 512 : (half + 1) * 512], dnh[half],
                    float(MASK_NUM), None, op0=mybir.AluOpType.add,
                )
            rrow = dvp.tile([1, N], F32, tag="rrow")
            nc.vector.reciprocal(rrow, rsum)
            w_i = nc.sync.dma_start(out=dscr[h, :], in_=rrow)
            rb_sb = dvp.tile([128, N], F32)
            r_i = nc.sync.dma_start(
                out=rb_sb, in_=dscr[h, :].partition_broadcast(128)
            )
            add_dep_helper(r_i.ins, w_i.ins, sync=True, reason="recip RAW")
            nc.vector.tensor_mul(outT_sb[:, h, :], outT_sb[:, h, :], rb_sb)

        emit_B(0)
        emit_A(0)
        emit_B(1)
        emit_A(1)
        for h in range(HPC):
            nxt = h + 2
            if nxt < HPC:
                ot, dnh = emit_C_mm(
                    h, interleave=lambda jt, h2=nxt: emit_A_part(h2, jt)
                )
                emit_B(nxt)
            else:
                ot, dnh = emit_C_mm(h)
            emit_C_norm(h, ot, dnh)

        for p in reversed(attn_pools):
            p.__exit__(None, None, None)
        vT_pool.__exit__(None, None, None)

        # ============= phase 3: to_out partial =============================
        with (
            tc.tile_pool(name="fo_psum", bufs=3, space="PSUM") as fo_psum,
            tc.tile_pool(name="fout", bufs=4) as fout_pool,
        ):
            def finish_oc(oc, fo, last=False):
                if not last:
                    for half in range(2):
                        nc.tensor.matmul(
                            fo[:, half * 512 : (half + 1) * 512],
                            woT_sb[:, HPC - 1, oc * 128 : (oc + 1) * 128],
                            outT_sb[:, HPC - 1,
                                    half * 512 : (half + 1) * 512],
                            start=False, stop=True,
                        )
                fout = fout_pool.tile([128, N], BF16)
                if last:
                    # quarter-split drain + store so the tail is short
                    engs = (nc.sync, nc.gpsimd, nc.sync, nc.gpsimd)
                    for q in range(4):
                        sl = slice(q * 256, (q + 1) * 256)
                        if q % 2 == 0:
                            nc.vector.tensor_copy(fout[:, sl], fo[:, sl])
                        else:
                            nc.scalar.activation(
                                out=fout[:, sl], in_=fo[:, sl], func=Ident
                            )
                        engs[q].dma_start(
                            out=outT_d[oc * 128 : (oc + 1) * 128, sl],
                            in_=fout[:, sl],
                        )
                    return
                if oc % 2 == 0:
                    nc.vector.tensor_copy(fout, fo)
                else:
                    nc.scalar.activation(out=fout, in_=fo, func=Ident)
                for sh in range(2):
                    eng = nc.sync if sh % 2 == 0 else nc.scalar
                    eng.dma_start(
                        out=outT_d[oc * 128 : (oc + 1) * 128,
                                   sh * 512 : (sh + 1) * 512],
                        in_=fout[:, sh * 512 : (sh + 1) * 512],
                    )

            pending_oc = None
            for oc in range(16):
                fo = fo_psum.tile([128, N], F32)
                if oc == 15 and pending_oc is not None:
                    # flush early so its drain overlaps the last matmuls
                    finish_oc(*pending_oc)
                    pending_oc = None
                nh = HPC if oc == 15 else HPC - 1
                for half in range(2):
                    for h in range(nh):
                        nc.tensor.matmul(
                            fo[:, half * 512 : (half + 1) * 512],
                            woT_sb[:, h, oc * 128 : (oc + 1) * 128],
                            outT_sb[:, h, half * 512 : (half + 1) * 512],
                            start=(h == 0), stop=(h == HPC - 1),
                        )
                if pending_oc is not None:
                    finish_oc(*pending_oc)
                pending_oc = (oc, fo)
            finish_oc(*pending_oc, last=True)


_CACHE = {}


def _get_module():
    if "nc" in _CACHE:
        return _CACHE["nc"]
    nc = bacc.Bacc("TRN2", target_bir_lowering=False, debug=False, num_devices=8)
    xt_d = nc.dram_tensor("xt", (HPC, 128, N), BF16, kind="ExternalInput").ap()
    xsum_d = nc.dram_tensor("xsum", (128, N), F32R, kind="ExternalInput").ap()
    wm_d = nc.dram_tensor("wm", (C, C), BF16, kind="ExternalInput").ap()
    wv_d = nc.dram_tensor("wvT", (C, C), BF16, kind="ExternalInput").ap()
    wke_d = nc.dram_tensor("wke", (C, 32), BF16, kind="ExternalInput").ap()
    eb_d = nc.dram_tensor("eb", (128, 1), F32, kind="ExternalInput").ap()
    bv_d = nc.dram_tensor("bv", (C, 1), F32, kind="ExternalInput").ap()
    wtc_d = nc.dram_tensor("wtc", (C, 1), F32R, kind="ExternalInput").ap()
    wo_d = nc.dram_tensor("woT", (HPC * C, D), BF16, kind="ExternalInput").ap()
    outT_d = nc.dram_tensor("outT", (D, N), BF16, kind="ExternalOutput").ap()

    with tile.TileContext(nc) as tc:
        _body(tc, xt_d, xsum_d, wm_d, wv_d, wke_d, eb_d, bv_d,
              wtc_d, wo_d, outT_d)
    nc.compile()
    _CACHE["nc"] = nc
    return nc


def make_in_maps(x, Wq, bq, Wk, bk, Wv, bv, Wl, bl, Wo, bo):
    x = np.asarray(x, np.float32)
    Wq = np.asarray(Wq, np.float32)
    Wk = np.asarray(Wk, np.float32)
    Wv = np.asarray(Wv, np.float32)
    Wl = np.asarray(Wl, np.float32)
    Wo = np.asarray(Wo, np.float32)
    bq_ = np.asarray(bq, np.float32)
    bk_ = np.asarray(bk, np.float32)
    we = (Wl[0] @ Wq) / float(NCHUNK)  # (128,) logits weight per chunk
    common = {
        # bias-folded attention: K2 = (Wq^T Wk) x, e = (Wk^T bq).x + bq.bk
        "wm": np.ascontiguousarray(Wk.T @ Wq).astype(ml_dtypes.bfloat16),
        "wvT": np.ascontiguousarray(Wv.T).astype(ml_dtypes.bfloat16),
        "wke": np.concatenate(
            [(Wk.T @ bq_).reshape(C, 1), np.zeros((C, 31), np.float32)],
            axis=1,
        ).astype(ml_dtypes.bfloat16),
        "eb": np.full((128, 1), float(bq_ @ bk_) * SCALE, np.float32),
        "bv": np.asarray(bv, np.float32).reshape(C, 1),
        "wtc": we.astype(np.float32).reshape(C, 1),
    }
    woT = np.ascontiguousarray(Wo.T)  # (d, o)
    woT_half = [
        woT[0:1024, :].astype(ml_dtypes.bfloat16),
        woT[1024:2048, :].astype(ml_dtypes.bfloat16),
    ]
    in_maps = []
    for core in range(8):
        b, g = divmod(core, 2)
        xtb = np.ascontiguousarray(x[b].T).reshape(NCHUNK, 128, N)
        xsum = xtb.sum(axis=0)  # (128, N) f32: chunk-summed x for logits
        own = xtb[g * 8 : g * 8 + 8].astype(ml_dtypes.bfloat16)
        in_maps.append({
            "xt": np.ascontiguousarray(own),
            "xsum": np.ascontiguousarray(xsum),
            "woT": woT_half[g],
            **common,
        })
    return in_maps


def run_spmd(in_maps, trace=False, **kw):
    nc = _get_module()
    return bass_utils.run_bass_kernel_spmd(
        nc, in_maps, core_ids=list(range(8)), trace=trace, **kw
    )


def gather(results, bo):
    bo = np.asarray(bo, np.float32)
    out = np.empty((B, N, D), np.float32)
    for b in range(B):
        p0 = results[2 * b]["outT"].astype(np.float32).T
        p1 = results[2 * b + 1]["outT"].astype(np.float32).T
        out[b] = p0 + p1 + bo
    return out


def kernel(x, Wq, bq, Wk, bk, Wv, bv, Wl, bl, Wo, bo, stage=None, **_unused):
    in_maps = make_in_maps(x, Wq, bq, Wk, bk, Wv, bv, Wl, bl, Wo, bo)
    try:
        res = run_spmd(in_maps)
    except Exception:
        # transient device/runtime hiccup: retry once after a short pause
        import time as _time

        _time.sleep(2.0)
        res = run_spmd(in_maps)
    return gather(res.results, bo)
